# revision 2
# baseline (speedup 1.0000x reference)
"""AttentionCell (Bahdanau attention + LSTM step) on 8 TRN2 NeuronCores.

Data-parallel over batch: B=256 rows sharded 32/core. Weights replicated.

Math per batch row b (T=256, IN=512, H=512, NE=96):
  proj_H  = batch_H @ W_i2h.T                       [T, H]
  proj_p  = prev_h @ W_h2h.T + b_h2h                [H]
  e       = tanh(proj_H + proj_p) @ W_score[0]      [T]
  alpha   = softmax(e)                              [T]
  context = alpha @ batch_H                         [IN]
  gates   = [context, onehot, 1] @ W_ihT_aug + prev_h @ W_hh.T
  i,f,g,o = split(gates); new_c = sig(f)*prev_c + sig(i)*tanh(g)
  new_h   = sig(o)*tanh(new_c)

On-chip layout strategy (per core, S=32 rows = 16 pairs of 2):
  - proj_H computed transposed ([H, T] per row-pair, 2 rows fused to N=512
    matmuls) so proj_p enters as the ACT per-partition bias of the tanh.
  - e via PE: lhsT = W_score column chunks, rhs = tanh tiles, accumulated
    over the 4 H-chunks into a [1, 512] PSUM row -> staged -> 2 rows of
    e_all [32, 256].
  - batched softmax over all 32 rows at once (rowwise over free dim).
  - context via PE: lhsT = alpha^T column, rhs = natural batch_H tiles.
  - LSTM gates with batch on PSUM partitions, bias folded in via an
    ones-row in the onehot^T tile and a bias row in W_ihT_aug.
Matmul operands in bf16 (fp32 PSUM accumulation); everything else fp32.
"""

import sys

sys.path.insert(0, "/opt/trn_rl_repo")

from contextlib import ExitStack

import ml_dtypes
import numpy as np

import concourse.bacc as bacc
import concourse.mybir as mybir
from concourse import masks
from concourse.bass_utils import run_bass_kernel_spmd
from concourse.tile import TileContext

F32 = mybir.dt.float32
BF16 = mybir.dt.bfloat16
AF = mybir.ActivationFunctionType
ALU = mybir.AluOpType
AX = mybir.AxisListType

B, T, IN, H, NE = 256, 256, 512, 512, 96
NCORES = 8
S = B // NCORES          # 32 batch rows per core
NP = S // 2              # 16 row-pairs per core
KI = IN // 128           # 4 contraction chunks over IN
KH = H // 128            # 4 chunks over H
KT = T // 128            # 2 chunks over T

_bf16 = ml_dtypes.bfloat16


def _build():
    nc = bacc.Bacc("TRN2", target_bir_lowering=False, debug=False,
                   num_devices=NCORES)
    d = {
        "bht":    nc.dram_tensor("bht", [NP, IN, 512], BF16, kind="ExternalInput"),
        "bhn":    nc.dram_tensor("bhn", [S, T, IN], BF16, kind="ExternalInput"),
        "prevht": nc.dram_tensor("prevht", [128, KH, S], BF16, kind="ExternalInput"),
        "prevc":  nc.dram_tensor("prevc", [S, H], F32, kind="ExternalInput"),
        "oh1t":   nc.dram_tensor("oh1t", [NE + 1, S], BF16, kind="ExternalInput"),
        "wi2ht":  nc.dram_tensor("wi2ht", [IN, H], BF16, kind="ExternalInput"),
        "wscore": nc.dram_tensor("wscore", [128, KH], BF16, kind="ExternalInput"),
        "wh2ht":  nc.dram_tensor("wh2ht", [H, H], BF16, kind="ExternalInput"),
        "bh2h":   nc.dram_tensor("bh2h", [128, KH], F32, kind="ExternalInput"),
        "wiht":   nc.dram_tensor("wiht", [IN + NE + 1, 4 * H], BF16, kind="ExternalInput"),
        "whht":   nc.dram_tensor("whht", [H, 4 * H], BF16, kind="ExternalInput"),
        "newh":   nc.dram_tensor("newh", [S, H], F32, kind="ExternalOutput"),
        "newc":   nc.dram_tensor("newc", [S, H], F32, kind="ExternalOutput"),
        "alpha":  nc.dram_tensor("alpha", [S, T], F32, kind="ExternalOutput"),
    }

    with TileContext(nc) as tc, ExitStack() as ctx:
        const = ctx.enter_context(tc.tile_pool(name="const", bufs=1))

        ident = const.tile([32, 32], F32)
        masks.make_identity(nc, ident[:])
        ident16 = const.tile([32, 32], BF16)
        masks.make_identity(nc, ident16[:])

        # Resident weights / state
        wi2ht = const.tile([128, KI, H], BF16)
        nc.sync.dma_start(out=wi2ht[:],
                          in_=d["wi2ht"].ap().rearrange("(k p) h -> p k h", p=128))
        wsc = const.tile([128, KH], BF16)
        nc.sync.dma_start(out=wsc[:], in_=d["wscore"].ap()[:])
        pht = const.tile([128, KH, S], BF16)
        nc.sync.dma_start(out=pht[:], in_=d["prevht"].ap()[:])
        oh1 = const.tile([NE + 1, S], BF16)
        nc.sync.dma_start(out=oh1[:], in_=d["oh1t"].ap()[:])
        bh2h = const.tile([128, KH], F32)
        nc.sync.dma_start(out=bh2h[:], in_=d["bh2h"].ap()[:])
        pc_sb = const.tile([S, H], F32)
        nc.sync.dma_start(out=pc_sb[:], in_=d["prevc"].ap()[:])

        ppt = const.tile([128, KH, S], F32)       # proj_prev^T (+ b_h2h)
        e_all = const.tile([S, T], F32)
        alpha_sb = const.tile([S, T], F32)
        alphat = const.tile([128, KT, S], BF16)   # alpha^T
        ctxnat = const.tile([S, IN], BF16)        # context rows
        inpt = const.tile([128, KI, S], BF16)     # context^T

        # ---- Phase A: proj_prev^T = (W_h2h @ prev_h.T) + b_h2h ----
        with tc.tile_pool(name="psA", bufs=2, space="PSUM") as psA, \
             tc.tile_pool(name="wA", bufs=2) as wA:
            ps_pp = psA.tile([S, H], F32, tag="pp")
            for k in range(KH):
                wt = wA.tile([128, H], BF16, tag="wh2h")
                nc.sync.dma_start(out=wt[:],
                                  in_=d["wh2ht"].ap()[k * 128:(k + 1) * 128, :])
                nc.tensor.matmul(ps_pp[:], pht[:, k, :], wt[:],
                                 start=(k == 0), stop=(k == KH - 1))
            pp_nat = const.tile([S, H], F32)
            nc.scalar.copy(pp_nat[:], ps_pp[:])
            for k in range(KH):
                ps_t = psA.tile([128, S], F32, tag="ppt")
                nc.tensor.transpose(ps_t[:], pp_nat[:, k * 128:(k + 1) * 128],
                                    ident[:])
                nc.vector.tensor_scalar_add(ppt[:, k, :], ps_t[:],
                                            bh2h[:, k:k + 1])

        # ---- Phase B: per row-pair: proj_H^T -> tanh -> e ----
        with tc.tile_pool(name="bhtP", bufs=2) as bhtP, \
             tc.tile_pool(name="thP", bufs=2) as thP, \
             tc.tile_pool(name="psB", bufs=4, space="PSUM") as psB, \
             tc.tile_pool(name="psE", bufs=2, space="PSUM") as psE, \
             tc.tile_pool(name="stgE", bufs=2) as stgE:
            for p in range(NP):
                bt = bhtP.tile([128, KI, 512], BF16, tag="bht")
                nc.sync.dma_start(
                    out=bt[:],
                    in_=d["bht"].ap()[p].rearrange("(k p) x -> p k x", p=128))
                ths = []
                for m in range(KH):
                    ps = psB.tile([128, 512], F32, tag="pj")
                    for k in range(KI):
                        nc.tensor.matmul(ps[:],
                                         wi2ht[:, k, m * 128:(m + 1) * 128],
                                         bt[:, k, :],
                                         start=(k == 0), stop=(k == KI - 1))
                    th = thP.tile([128, 512], BF16, tag=f"th{m}")
                    for h in range(2):
                        bidx = 2 * p + h
                        nc.scalar.activation(th[:, h * 256:(h + 1) * 256],
                                             ps[:, h * 256:(h + 1) * 256],
                                             AF.Tanh,
                                             bias=ppt[:, m, bidx:bidx + 1],
                                             scale=1.0)
                    ths.append(th)
                pe = psE.tile([1, 512], F32, tag="e")
                for m in range(KH):
                    nc.tensor.matmul(pe[:], wsc[:, m:m + 1], ths[m][:],
                                     start=(m == 0), stop=(m == KH - 1))
                stg = stgE.tile([1, 512], F32, tag="estg")
                nc.scalar.copy(stg[:], pe[:])
                nc.sync.dma_start(out=e_all[2 * p:2 * p + 2, :], in_=stg[:])

        # ---- Phase C: batched softmax + alpha^T ----
        negmax = const.tile([S, 1], F32)
        nc.vector.tensor_reduce(negmax[:], e_all[:], axis=AX.XYZW,
                                op=ALU.max, negate=True)
        alphaU = const.tile([S, T], F32)
        esum = const.tile([S, 1], F32)
        nc.scalar.activation(alphaU[:], e_all[:], AF.Exp, bias=negmax[:],
                             scale=1.0, accum_out=esum[:])
        rsum = const.tile([S, 1], F32)
        nc.vector.reciprocal(rsum[:], esum[:])
        nc.scalar.activation(alpha_sb[:], alphaU[:], AF.Copy, bias=0.0,
                             scale=rsum[:])
        nc.sync.dma_start(out=d["alpha"].ap()[:], in_=alpha_sb[:])
        with tc.tile_pool(name="psC", bufs=2, space="PSUM") as psC:
            for kt in range(KT):
                pt = psC.tile([128, S], F32, tag="alt")
                nc.tensor.transpose(pt[:], alpha_sb[:, kt * 128:(kt + 1) * 128],
                                    ident[:])
                nc.scalar.copy(alphat[:, kt, :], pt[:])

        # ---- Phase D: context = alpha @ batch_H ----
        with tc.tile_pool(name="bhnP", bufs=8) as bhnP, \
             tc.tile_pool(name="psD", bufs=4, space="PSUM") as psD, \
             tc.tile_pool(name="stgD", bufs=4) as stgD:
            for b in range(S):
                bn = bhnP.tile([128, KT, IN], BF16, tag="bhn")
                nc.sync.dma_start(
                    out=bn[:],
                    in_=d["bhn"].ap()[b].rearrange("(k p) x -> p k x", p=128))
                ps_c = psD.tile([1, IN], F32, tag="ctx")
                for kt in range(KT):
                    nc.tensor.matmul(ps_c[:], alphat[:, kt, b:b + 1],
                                     bn[:, kt, :],
                                     start=(kt == 0), stop=(kt == KT - 1))
                cs = stgD.tile([1, IN], BF16, tag="cstg")
                nc.scalar.copy(cs[:], ps_c[:])
                nc.sync.dma_start(out=ctxnat[b:b + 1, :], in_=cs[:])
            with tc.tile_pool(name="psD2", bufs=2, space="PSUM") as psD2:
                for k in range(KI):
                    pt = psD2.tile([128, S], BF16, tag="ctxT")
                    nc.tensor.transpose(pt[:], ctxnat[:, k * 128:(k + 1) * 128],
                                        ident16[:])
                    nc.scalar.copy(inpt[:, k, :], pt[:])

        # ---- Phase E: LSTM gates + cell update ----
        with tc.tile_pool(name="wE", bufs=6) as wE, \
             tc.tile_pool(name="psG", bufs=4, space="PSUM") as psG, \
             tc.tile_pool(name="lst", bufs=1) as lst:
            gate_ps = []
            for n in range(4):
                ps_g = psG.tile([S, 512], F32, tag="g")
                for k in range(KI):
                    wt = wE.tile([128, 512], BF16, tag="wg")
                    nc.sync.dma_start(
                        out=wt[:],
                        in_=d["wiht"].ap()[k * 128:(k + 1) * 128,
                                           n * 512:(n + 1) * 512])
                    nc.tensor.matmul(ps_g[:], inpt[:, k, :], wt[:],
                                     start=(k == 0), stop=False)
                wt97 = wE.tile([NE + 1, 512], BF16, tag="wg97")
                nc.sync.dma_start(out=wt97[:],
                                  in_=d["wiht"].ap()[IN:IN + NE + 1,
                                                     n * 512:(n + 1) * 512])
                nc.tensor.matmul(ps_g[:], oh1[:], wt97[:],
                                 start=False, stop=False)
                for k in range(KH):
                    wt2 = wE.tile([128, 512], BF16, tag="wg")
                    nc.sync.dma_start(
                        out=wt2[:],
                        in_=d["whht"].ap()[k * 128:(k + 1) * 128,
                                           n * 512:(n + 1) * 512])
                    nc.tensor.matmul(ps_g[:], pht[:, k, :], wt2[:],
                                     start=False, stop=(k == KH - 1))
                gate_ps.append(ps_g)

            i_s = lst.tile([S, 512], F32, tag="i_s")
            f_s = lst.tile([S, 512], F32, tag="f_s")
            g_t = lst.tile([S, 512], F32, tag="g_t")
            o_s = lst.tile([S, 512], F32, tag="o_s")
            nc.scalar.activation(i_s[:], gate_ps[0][:], AF.Sigmoid)
            nc.scalar.activation(f_s[:], gate_ps[1][:], AF.Sigmoid)
            nc.scalar.activation(g_t[:], gate_ps[2][:], AF.Tanh)
            nc.scalar.activation(o_s[:], gate_ps[3][:], AF.Sigmoid)
            t1 = lst.tile([S, 512], F32, tag="t1")
            t2 = lst.tile([S, 512], F32, tag="t2")
            newc = lst.tile([S, 512], F32, tag="newc")
            nc.vector.tensor_mul(t1[:], f_s[:], pc_sb[:])
            nc.vector.tensor_mul(t2[:], i_s[:], g_t[:])
            nc.vector.tensor_add(newc[:], t1[:], t2[:])
            nc.sync.dma_start(out=d["newc"].ap()[:], in_=newc[:])
            tcn = lst.tile([S, 512], F32, tag="tcn")
            nc.scalar.activation(tcn[:], newc[:], AF.Tanh)
            newh = lst.tile([S, 512], F32, tag="newh")
            nc.vector.tensor_mul(newh[:], o_s[:], tcn[:])
            nc.sync.dma_start(out=d["newh"].ap()[:], in_=newh[:])

    nc.compile()
    return nc


_NC_CACHE = None


def _get_nc():
    global _NC_CACHE
    if _NC_CACHE is None:
        _NC_CACHE = _build()
    return _NC_CACHE


def _prep_inputs(prev_h, prev_c, batch_H, char_onehots,
                 W_i2h, W_h2h, b_h2h, W_score, W_ih, W_hh, b_ih, b_hh):
    """Host-side sharding + layout transforms. Returns list of per-core dicts."""
    f32 = np.float32
    bht_all = np.ascontiguousarray(
        batch_H.astype(_bf16).reshape(NCORES, NP, 2, T, IN)
        .transpose(0, 1, 4, 2, 3).reshape(NCORES, NP, IN, 512))
    bhn_all = batch_H.astype(_bf16).reshape(NCORES, S, T, IN)
    prevht_all = np.ascontiguousarray(
        prev_h.astype(_bf16).reshape(NCORES, S, KH, 128).transpose(0, 3, 2, 1))
    prevc_all = prev_c.astype(f32).reshape(NCORES, S, H)
    ones = np.ones((NCORES, 1, S), _bf16)
    oh1t_all = np.concatenate(
        [np.ascontiguousarray(
            char_onehots.astype(_bf16).reshape(NCORES, S, NE).transpose(0, 2, 1)),
         ones], axis=1)

    wi2ht = np.ascontiguousarray(W_i2h.T).astype(_bf16)
    wscore = np.ascontiguousarray(W_score[0].reshape(KH, 128).T).astype(_bf16)
    wh2ht = np.ascontiguousarray(W_h2h.T).astype(_bf16)
    bh2h_c = np.ascontiguousarray(b_h2h.reshape(KH, 128).T).astype(f32)
    wiht = np.concatenate(
        [np.ascontiguousarray(W_ih[:, :IN].T),
         np.ascontiguousarray(W_ih[:, IN:].T),
         (b_ih + b_hh)[None, :]], axis=0).astype(_bf16)
    whht = np.ascontiguousarray(W_hh.T).astype(_bf16)

    return [{
        "bht": np.ascontiguousarray(bht_all[c]),
        "bhn": np.ascontiguousarray(bhn_all[c]),
        "prevht": np.ascontiguousarray(prevht_all[c]),
        "prevc": np.ascontiguousarray(prevc_all[c]),
        "oh1t": np.ascontiguousarray(oh1t_all[c]),
        "wi2ht": wi2ht,
        "wscore": wscore,
        "wh2ht": wh2ht,
        "bh2h": bh2h_c,
        "wiht": wiht,
        "whht": whht,
    } for c in range(NCORES)]


def _run(inputs, trace=False):
    nc = _get_nc()
    in_maps = _prep_inputs(**{k: np.asarray(v) for k, v in inputs.items()})
    res = run_bass_kernel_spmd(nc, in_maps, core_ids=list(range(NCORES)),
                               trace=trace)
    new_h = np.concatenate([res.results[c]["newh"] for c in range(NCORES)], 0)
    new_c = np.concatenate([res.results[c]["newc"] for c in range(NCORES)], 0)
    alpha = np.concatenate([res.results[c]["alpha"] for c in range(NCORES)], 0)
    return (new_h.astype(np.float32), new_c.astype(np.float32),
            alpha.astype(np.float32)[:, :, None]), res


def kernel(**inputs):
    out, _ = _run(inputs, trace=False)
    return out


# revision 4
# speedup vs baseline: 1.1127x; 1.1127x over previous
"""AttentionCell (Bahdanau attention + LSTM step) on 8 TRN2 NeuronCores.

Data-parallel over batch: B=256 rows sharded 32/core. Weights replicated.

Math per batch row b (T=256, IN=512, H=512, NE=96):
  proj_H  = batch_H @ W_i2h.T                       [T, H]
  proj_p  = prev_h @ W_h2h.T + b_h2h                [H]
  e       = tanh(proj_H + proj_p) @ W_score[0]      [T]
  alpha   = softmax(e)                              [T]
  context = alpha @ batch_H                         [IN]
  gates   = [context, onehot, 1] @ W_ihT_aug + prev_h @ W_hh.T
  i,f,g,o = split(gates); new_c = sig(f)*prev_c + sig(i)*tanh(g)
  new_h   = sig(o)*tanh(new_c)

Fully streamed per row-PAIR (2 batch rows fused into N=512 matmuls):
  proj^T on PE (lhsT = W_i2h^T tiles, rhs = batch_H^T tiles) -> tanh on ACT
  with proj_prev as the per-partition bias -> e on PE (lhsT = W_score
  chunks) -> per-pair softmax (DVE/ACT) -> alpha broadcast to 128
  partitions via a K=2 selector matmul on PE -> context on DVE as
  (batch_H^T * alpha_bcast) multiplied then reduced over T, accumulating
  context^T columns directly -> LSTM gates on PE with batch rows on PSUM
  partitions and biases folded in via an ones-row.
Matmul operands bf16 (fp32 PSUM accumulation); everything else fp32.
"""

import sys

sys.path.insert(0, "/opt/trn_rl_repo")

from contextlib import ExitStack

import ml_dtypes
import numpy as np

import concourse.bacc as bacc
import concourse.mybir as mybir
from concourse import masks
from concourse.bass_utils import run_bass_kernel_spmd
from concourse.tile import TileContext

F32 = mybir.dt.float32
BF16 = mybir.dt.bfloat16
AF = mybir.ActivationFunctionType
ALU = mybir.AluOpType
AX = mybir.AxisListType

B, T, IN, H, NE = 256, 256, 512, 512, 96
NCORES = 8
S = B // NCORES          # 32 batch rows per core
NP = S // 2              # 16 row-pairs per core
KI = IN // 128           # 4 contraction chunks over IN
KH = H // 128            # 4 chunks over H

_bf16 = ml_dtypes.bfloat16


def _build():
    nc = bacc.Bacc("TRN2", target_bir_lowering=False, debug=False,
                   num_devices=NCORES)
    d = {
        "bht":    nc.dram_tensor("bht", [NP, IN, 512], BF16, kind="ExternalInput"),
        "prevht": nc.dram_tensor("prevht", [128, KH, S], BF16, kind="ExternalInput"),
        "prevc":  nc.dram_tensor("prevc", [S, H], F32, kind="ExternalInput"),
        "oh1t":   nc.dram_tensor("oh1t", [NE + 1, S], BF16, kind="ExternalInput"),
        "wi2ht":  nc.dram_tensor("wi2ht", [IN, H], BF16, kind="ExternalInput"),
        "wscore": nc.dram_tensor("wscore", [128, KH], BF16, kind="ExternalInput"),
        "wh2ht":  nc.dram_tensor("wh2ht", [H, H], BF16, kind="ExternalInput"),
        "bh2h":   nc.dram_tensor("bh2h", [128, KH], F32, kind="ExternalInput"),
        "wiht":   nc.dram_tensor("wiht", [IN + NE + 1, 4 * H], BF16, kind="ExternalInput"),
        "whht":   nc.dram_tensor("whht", [H, 4 * H], BF16, kind="ExternalInput"),
        "newh":   nc.dram_tensor("newh", [S, H], F32, kind="ExternalOutput"),
        "newc":   nc.dram_tensor("newc", [S, H], F32, kind="ExternalOutput"),
        "alpha":  nc.dram_tensor("alpha", [S, T], F32, kind="ExternalOutput"),
    }
    # K=2 selector for broadcasting a [2, 256] alpha pair into the two
    # 256-col halves of a [128, 512] tile.
    sel_np = np.zeros((2, 256), _bf16)
    sel_np[0, :128] = 1
    sel_np[1, 128:] = 1
    d_sel = nc.inline_tensor(sel_np, name="selc")

    with TileContext(nc) as tc, ExitStack() as ctx:
        const = ctx.enter_context(tc.tile_pool(name="const", bufs=1))

        ident = const.tile([32, 32], F32)
        masks.make_identity(nc, ident[:])

        sel = const.tile([2, 256], BF16)
        nc.sync.dma_start(out=sel[:], in_=d_sel.ap()[:])
        wi2ht = const.tile([128, KI, H], BF16)
        nc.sync.dma_start(out=wi2ht[:],
                          in_=d["wi2ht"].ap().rearrange("(k p) h -> p k h", p=128))
        wsc = const.tile([128, KH], BF16)
        nc.sync.dma_start(out=wsc[:], in_=d["wscore"].ap()[:])
        pht = const.tile([128, KH, S], BF16)
        nc.sync.dma_start(out=pht[:], in_=d["prevht"].ap()[:])
        oh1 = const.tile([NE + 1, S], BF16)
        nc.sync.dma_start(out=oh1[:], in_=d["oh1t"].ap()[:])
        bh2h = const.tile([128, KH], F32)
        nc.sync.dma_start(out=bh2h[:], in_=d["bh2h"].ap()[:])
        pc_sb = const.tile([S, H], F32)
        nc.sync.dma_start(out=pc_sb[:], in_=d["prevc"].ap()[:])

        ppt = const.tile([128, KH, S], F32)       # proj_prev^T (+ b_h2h)
        ctxt = const.tile([128, KI, S], F32)      # context^T accumulator
        inpt = const.tile([128, KI, S], BF16)     # context^T bf16

        # ---- Phase A: proj_prev^T = (W_h2h @ prev_h.T) + b_h2h ----
        with tc.tile_pool(name="psA", bufs=2, space="PSUM") as psA, \
             tc.tile_pool(name="wA", bufs=2) as wA:
            ps_pp = psA.tile([S, H], F32, tag="pp")
            for k in range(KH):
                wt = wA.tile([128, H], BF16, tag="wh2h")
                nc.sync.dma_start(out=wt[:],
                                  in_=d["wh2ht"].ap()[k * 128:(k + 1) * 128, :])
                nc.tensor.matmul(ps_pp[:], pht[:, k, :], wt[:],
                                 start=(k == 0), stop=(k == KH - 1))
            pp_nat = const.tile([S, H], F32)
            nc.scalar.copy(pp_nat[:], ps_pp[:])
            for k in range(KH):
                ps_t = psA.tile([128, S], F32, tag="ppt")
                nc.tensor.transpose(ps_t[:], pp_nat[:, k * 128:(k + 1) * 128],
                                    ident[:])
                nc.vector.tensor_scalar_add(ppt[:, k, :], ps_t[:],
                                            bh2h[:, k:k + 1])

        # ---- Phase B: per row-pair, fully streamed ----
        with tc.tile_pool(name="bhtP", bufs=3) as bhtP, \
             tc.tile_pool(name="thP", bufs=2) as thP, \
             tc.tile_pool(name="smP", bufs=3) as smP, \
             tc.tile_pool(name="tmpP", bufs=2) as tmpP, \
             tc.tile_pool(name="psB", bufs=4, space="PSUM") as psB, \
             tc.tile_pool(name="psE", bufs=2, space="PSUM") as psE, \
             tc.tile_pool(name="psBC", bufs=2, space="PSUM") as psBC:
            for p in range(NP):
                bt = bhtP.tile([128, KI, 512], BF16, tag="bht")
                nc.sync.dma_start(
                    out=bt[:],
                    in_=d["bht"].ap()[p].rearrange("(k p) x -> p k x", p=128))
                ths = []
                for m in range(KH):
                    ps = psB.tile([128, 512], F32, tag="pj")
                    for k in range(KI):
                        nc.tensor.matmul(ps[:],
                                         wi2ht[:, k, m * 128:(m + 1) * 128],
                                         bt[:, k, :],
                                         start=(k == 0), stop=(k == KI - 1))
                    th = thP.tile([128, 512], BF16, tag=f"th{m}")
                    for h in range(2):
                        bidx = 2 * p + h
                        nc.scalar.activation(th[:, h * 256:(h + 1) * 256],
                                             ps[:, h * 256:(h + 1) * 256],
                                             AF.Tanh,
                                             bias=ppt[:, m, bidx:bidx + 1],
                                             scale=1.0)
                    ths.append(th)
                pe = psE.tile([1, 512], F32, tag="e")
                for m in range(KH):
                    nc.tensor.matmul(pe[:], wsc[:, m:m + 1], ths[m][:],
                                     start=(m == 0), stop=(m == KH - 1))
                # stage e row -> [2, 256] rows, softmax per pair
                es = smP.tile([1, 512], F32, tag="es")
                nc.vector.tensor_copy(es[:], pe[:])
                ep = smP.tile([2, 256], F32, tag="ep")
                nc.sync.dma_start(out=ep[:], in_=es[:])
                negmax = smP.tile([2, 1], F32, tag="negmax")
                nc.vector.tensor_reduce(negmax[:], ep[:], axis=AX.XYZW,
                                        op=ALU.max, negate=True)
                expd = smP.tile([2, 256], F32, tag="expd")
                esum = smP.tile([2, 1], F32, tag="esum")
                nc.scalar.activation(expd[:], ep[:], AF.Exp, bias=negmax[:],
                                     scale=1.0, accum_out=esum[:])
                rsum = smP.tile([2, 1], F32, tag="rsum")
                nc.vector.reciprocal(rsum[:], esum[:])
                anb = smP.tile([2, 256], BF16, tag="anb")
                nc.vector.tensor_scalar_mul(anb[:], expd[:], rsum[:])
                af32 = smP.tile([2, 256], F32, tag="af32")
                nc.vector.tensor_scalar_mul(af32[:], expd[:], rsum[:])
                nc.sync.dma_start(out=d["alpha"].ap()[2 * p:2 * p + 2, :],
                                  in_=af32[:])
                # broadcast alpha pair to 128 partitions: [128, 2b x 256]
                pbc = psBC.tile([128, 512], F32, tag="pbc")
                nc.tensor.matmul(pbc[:, 0:256], sel[:, 0:128], anb[:],
                                 start=True, stop=True)
                nc.tensor.matmul(pbc[:, 256:512], sel[:, 128:256], anb[:],
                                 start=True, stop=True)
                pbc16 = tmpP.tile([128, 512], BF16, tag="pbc16")
                nc.vector.tensor_copy(pbc16[:], pbc[:])
                # context^T: multiply all 4 IN-chunks by alpha, reduce over T
                tmp = tmpP.tile([128, KI, 512], BF16, tag="ctmp")
                nc.vector.tensor_mul(
                    tmp[:], bt[:],
                    pbc16[:].rearrange("p (o x) -> p o x", o=1)
                         .to_broadcast((128, KI, 512)))
                nc.vector.tensor_reduce(
                    ctxt[:, :, 2 * p:2 * p + 2],
                    tmp[:].rearrange("p k (h t) -> p k h t", h=2),
                    axis=AX.X, op=ALU.add)

        # ---- Phase E: LSTM gates + cell update ----
        nc.vector.tensor_copy(inpt[:], ctxt[:])
        with tc.tile_pool(name="wE", bufs=3) as wE, \
             tc.tile_pool(name="psG", bufs=4, space="PSUM") as psG, \
             tc.tile_pool(name="lst", bufs=1) as lst:
            gate_ps = []
            for n in range(4):
                ps_g = psG.tile([S, 512], F32, tag="g")
                wta = wE.tile([128, KI, 512], BF16, tag="wga")
                nc.sync.dma_start(
                    out=wta[:],
                    in_=d["wiht"].ap()[0:IN, n * 512:(n + 1) * 512]
                        .rearrange("(k p) x -> p k x", p=128))
                wtb = wE.tile([NE + 1, 512], BF16, tag="wgb")
                nc.sync.dma_start(out=wtb[:],
                                  in_=d["wiht"].ap()[IN:IN + NE + 1,
                                                     n * 512:(n + 1) * 512])
                wtc = wE.tile([128, KH, 512], BF16, tag="wgc")
                nc.sync.dma_start(
                    out=wtc[:],
                    in_=d["whht"].ap()[:, n * 512:(n + 1) * 512]
                        .rearrange("(k p) x -> p k x", p=128))
                for k in range(KI):
                    nc.tensor.matmul(ps_g[:], inpt[:, k, :], wta[:, k, :],
                                     start=(k == 0), stop=False)
                nc.tensor.matmul(ps_g[:], oh1[:], wtb[:],
                                 start=False, stop=False)
                for k in range(KH):
                    nc.tensor.matmul(ps_g[:], pht[:, k, :], wtc[:, k, :],
                                     start=False, stop=(k == KH - 1))
                gate_ps.append(ps_g)

            i_s = lst.tile([S, 512], F32, tag="i_s")
            f_s = lst.tile([S, 512], F32, tag="f_s")
            g_t = lst.tile([S, 512], F32, tag="g_t")
            o_s = lst.tile([S, 512], F32, tag="o_s")
            nc.scalar.activation(i_s[:], gate_ps[0][:], AF.Sigmoid)
            nc.scalar.activation(f_s[:], gate_ps[1][:], AF.Sigmoid)
            nc.scalar.activation(g_t[:], gate_ps[2][:], AF.Tanh)
            nc.scalar.activation(o_s[:], gate_ps[3][:], AF.Sigmoid)
            t1 = lst.tile([S, 512], F32, tag="t1")
            t2 = lst.tile([S, 512], F32, tag="t2")
            newc = lst.tile([S, 512], F32, tag="newc")
            nc.vector.tensor_mul(t1[:], f_s[:], pc_sb[:])
            nc.vector.tensor_mul(t2[:], i_s[:], g_t[:])
            nc.vector.tensor_add(newc[:], t1[:], t2[:])
            nc.sync.dma_start(out=d["newc"].ap()[:], in_=newc[:])
            tcn = lst.tile([S, 512], F32, tag="tcn")
            nc.scalar.activation(tcn[:], newc[:], AF.Tanh)
            newh = lst.tile([S, 512], F32, tag="newh")
            nc.vector.tensor_mul(newh[:], o_s[:], tcn[:])
            nc.sync.dma_start(out=d["newh"].ap()[:], in_=newh[:])

    nc.compile()
    return nc


_NC_CACHE = None


def _get_nc():
    global _NC_CACHE
    if _NC_CACHE is None:
        _NC_CACHE = _build()
    return _NC_CACHE


def _prep_inputs(prev_h, prev_c, batch_H, char_onehots,
                 W_i2h, W_h2h, b_h2h, W_score, W_ih, W_hh, b_ih, b_hh):
    """Host-side sharding + layout transforms. Returns list of per-core dicts."""
    f32 = np.float32
    bht_all = np.ascontiguousarray(
        batch_H.astype(_bf16).reshape(NCORES, NP, 2, T, IN)
        .transpose(0, 1, 4, 2, 3).reshape(NCORES, NP, IN, 512))
    prevht_all = np.ascontiguousarray(
        prev_h.astype(_bf16).reshape(NCORES, S, KH, 128).transpose(0, 3, 2, 1))
    prevc_all = prev_c.astype(f32).reshape(NCORES, S, H)
    ones = np.ones((NCORES, 1, S), _bf16)
    oh1t_all = np.concatenate(
        [np.ascontiguousarray(
            char_onehots.astype(_bf16).reshape(NCORES, S, NE).transpose(0, 2, 1)),
         ones], axis=1)

    wi2ht = np.ascontiguousarray(W_i2h.T).astype(_bf16)
    wscore = np.ascontiguousarray(W_score[0].reshape(KH, 128).T).astype(_bf16)
    wh2ht = np.ascontiguousarray(W_h2h.T).astype(_bf16)
    bh2h_c = np.ascontiguousarray(b_h2h.reshape(KH, 128).T).astype(f32)
    wiht = np.concatenate(
        [np.ascontiguousarray(W_ih[:, :IN].T),
         np.ascontiguousarray(W_ih[:, IN:].T),
         (b_ih + b_hh)[None, :]], axis=0).astype(_bf16)
    whht = np.ascontiguousarray(W_hh.T).astype(_bf16)

    return [{
        "bht": np.ascontiguousarray(bht_all[c]),
        "prevht": np.ascontiguousarray(prevht_all[c]),
        "prevc": np.ascontiguousarray(prevc_all[c]),
        "oh1t": np.ascontiguousarray(oh1t_all[c]),
        "wi2ht": wi2ht,
        "wscore": wscore,
        "wh2ht": wh2ht,
        "bh2h": bh2h_c,
        "wiht": wiht,
        "whht": whht,
    } for c in range(NCORES)]


def _run(inputs, trace=False):
    nc = _get_nc()
    in_maps = _prep_inputs(**{k: np.asarray(v) for k, v in inputs.items()})
    res = run_bass_kernel_spmd(nc, in_maps, core_ids=list(range(NCORES)),
                               trace=trace)
    new_h = np.concatenate([res.results[c]["newh"] for c in range(NCORES)], 0)
    new_c = np.concatenate([res.results[c]["newc"] for c in range(NCORES)], 0)
    alpha = np.concatenate([res.results[c]["alpha"] for c in range(NCORES)], 0)
    return (new_h.astype(np.float32), new_c.astype(np.float32),
            alpha.astype(np.float32)[:, :, None]), res


def kernel(**inputs):
    out, _ = _run(inputs, trace=False)
    return out


# revision 6
# speedup vs baseline: 1.2656x; 1.1373x over previous
"""AttentionCell (Bahdanau attention + LSTM step) on 8 TRN2 NeuronCores.

Data-parallel over batch: B=256 rows sharded 32/core. Weights replicated.

Math per batch row b (T=256, IN=512, H=512, NE=96):
  proj_H  = batch_H @ W_i2h.T                       [T, H]
  proj_p  = prev_h @ W_h2h.T + b_h2h                [H]
  e       = tanh(proj_H + proj_p) @ W_score[0]      [T]
  alpha   = softmax(e)                              [T]
  context = alpha @ batch_H                         [IN]
  gates   = [context, onehot, 1] @ W_ihT_aug + prev_h @ W_hh.T
  i,f,g,o = split(gates); new_c = sig(f)*prev_c + sig(i)*tanh(g)
  new_h   = sig(o)*tanh(new_c)

Fully streamed per row-PAIR (2 batch rows fused into N=512 matmuls):
  proj^T on PE (lhsT = W_i2h^T tiles, rhs = batch_H^T tiles) -> tanh on ACT
  with proj_prev as the per-partition bias -> e on PE (lhsT = W_score
  chunks) -> per-pair softmax (DVE/ACT) -> alpha broadcast to 128
  partitions via a K=2 selector matmul on PE -> context on DVE as
  (batch_H^T * alpha_bcast) multiplied then reduced over T, accumulating
  context^T columns directly -> LSTM gates on PE with batch rows on PSUM
  partitions and biases folded in via an ones-row.
Matmul operands bf16 (fp32 PSUM accumulation); everything else fp32.
"""

import sys

sys.path.insert(0, "/opt/trn_rl_repo")

from contextlib import ExitStack

import ml_dtypes
import numpy as np

import concourse.bacc as bacc
import concourse.mybir as mybir
from concourse import masks
from concourse.bass_utils import run_bass_kernel_spmd
from concourse.tile import TileContext

F32 = mybir.dt.float32
BF16 = mybir.dt.bfloat16
AF = mybir.ActivationFunctionType
ALU = mybir.AluOpType
AX = mybir.AxisListType

B, T, IN, H, NE = 256, 256, 512, 512, 96
NCORES = 8
S = B // NCORES          # 32 batch rows per core
NP = S // 2              # 16 row-pairs per core
KI = IN // 128           # 4 contraction chunks over IN
KH = H // 128            # 4 chunks over H

_bf16 = ml_dtypes.bfloat16


def _build():
    nc = bacc.Bacc("TRN2", target_bir_lowering=False, debug=False,
                   num_devices=NCORES)
    d = {
        "bht":    nc.dram_tensor("bht", [NP, IN, 512], BF16, kind="ExternalInput"),
        "prevht": nc.dram_tensor("prevht", [128, KH, S], BF16, kind="ExternalInput"),
        "prevc":  nc.dram_tensor("prevc", [S, H], F32, kind="ExternalInput"),
        "oh1t":   nc.dram_tensor("oh1t", [NE + 1, S], BF16, kind="ExternalInput"),
        "wi2ht":  nc.dram_tensor("wi2ht", [IN, H], BF16, kind="ExternalInput"),
        "wscore": nc.dram_tensor("wscore", [128, KH], BF16, kind="ExternalInput"),
        "wh2ht":  nc.dram_tensor("wh2ht", [H, H], BF16, kind="ExternalInput"),
        "bh2h":   nc.dram_tensor("bh2h", [128, KH], F32, kind="ExternalInput"),
        "wiht":   nc.dram_tensor("wiht", [IN + NE + 1, 4 * H], BF16, kind="ExternalInput"),
        "whht":   nc.dram_tensor("whht", [H, 4 * H], BF16, kind="ExternalInput"),
        "newh":   nc.dram_tensor("newh", [S, H], F32, kind="ExternalOutput"),
        "newc":   nc.dram_tensor("newc", [S, H], F32, kind="ExternalOutput"),
        "alpha":  nc.dram_tensor("alpha", [S, T], F32, kind="ExternalOutput"),
    }
    # K=2 selector for broadcasting a [2, 256] alpha pair into the two
    # 256-col halves of a [128, 512] tile.
    sel_np = np.zeros((2, 256), _bf16)
    sel_np[0, :128] = 1
    sel_np[1, 128:] = 1
    d_sel = nc.inline_tensor(sel_np, name="selc")

    with TileContext(nc) as tc, ExitStack() as ctx:
        const = ctx.enter_context(tc.tile_pool(name="const", bufs=1))

        ident = const.tile([32, 32], F32)
        masks.make_identity(nc, ident[:])

        sel = const.tile([2, 256], BF16)
        nc.sync.dma_start(out=sel[:], in_=d_sel.ap()[:])
        wi2ht = const.tile([128, KI, H], BF16)
        nc.sync.dma_start(out=wi2ht[:],
                          in_=d["wi2ht"].ap().rearrange("(k p) h -> p k h", p=128))
        wsc = const.tile([128, KH], BF16)
        nc.sync.dma_start(out=wsc[:], in_=d["wscore"].ap()[:])
        pht = const.tile([128, KH, S], BF16)
        nc.sync.dma_start(out=pht[:], in_=d["prevht"].ap()[:])
        oh1 = const.tile([NE + 1, S], BF16)
        nc.sync.dma_start(out=oh1[:], in_=d["oh1t"].ap()[:])
        bh2h = const.tile([128, KH], F32)
        nc.sync.dma_start(out=bh2h[:], in_=d["bh2h"].ap()[:])
        pc_sb = const.tile([S, H], F32)
        nc.sync.dma_start(out=pc_sb[:], in_=d["prevc"].ap()[:])

        ppt = const.tile([128, KH, S], F32)       # proj_prev^T (+ b_h2h)
        ctxt = const.tile([128, KI, S], F32)      # context^T accumulator
        inpt = const.tile([128, KI, S], BF16)     # context^T bf16

        # ---- Phase A: proj_prev^T = (W_h2h @ prev_h.T) + b_h2h ----
        with tc.tile_pool(name="psA", bufs=2, space="PSUM") as psA, \
             tc.tile_pool(name="wA", bufs=2) as wA:
            ps_pp = psA.tile([S, H], F32, tag="pp")
            for k in range(KH):
                wt = wA.tile([128, H], BF16, tag="wh2h")
                nc.sync.dma_start(out=wt[:],
                                  in_=d["wh2ht"].ap()[k * 128:(k + 1) * 128, :])
                nc.tensor.matmul(ps_pp[:], pht[:, k, :], wt[:],
                                 start=(k == 0), stop=(k == KH - 1))
            pp_nat = const.tile([S, H], F32)
            nc.scalar.copy(pp_nat[:], ps_pp[:])
            for k in range(KH):
                ps_t = psA.tile([128, S], F32, tag="ppt")
                nc.tensor.transpose(ps_t[:], pp_nat[:, k * 128:(k + 1) * 128],
                                    ident[:])
                nc.vector.tensor_scalar_add(ppt[:, k, :], ps_t[:],
                                            bh2h[:, k:k + 1])

        # ---- Phase B: per row-pair, fully streamed ----
        with tc.tile_pool(name="bhtP", bufs=4) as bhtP, \
             tc.tile_pool(name="thP", bufs=2) as thP, \
             tc.tile_pool(name="smP", bufs=3) as smP, \
             tc.tile_pool(name="tmpP", bufs=2) as tmpP, \
             tc.tile_pool(name="psB", bufs=4, space="PSUM") as psB, \
             tc.tile_pool(name="psE", bufs=2, space="PSUM") as psE, \
             tc.tile_pool(name="psBC", bufs=2, space="PSUM") as psBC:
            # Software-pipelined: front(p) = proj/tanh/e/softmax; back(p-1) =
            # alpha-broadcast + context. Shifting the back half by one pair
            # keeps the PE stream free of softmax-latency bubbles.
            bts = {}
            anbs = {}

            def front(p):
                bt = bhtP.tile([128, KI, 512], BF16, tag="bht")
                bts[p] = bt
                nc.sync.dma_start(
                    out=bt[:],
                    in_=d["bht"].ap()[p].rearrange("(k p) x -> p k x", p=128))
                ths = []
                for m in range(KH):
                    ps = psB.tile([128, 512], F32, tag="pj")
                    for k in range(KI):
                        nc.tensor.matmul(ps[:],
                                         wi2ht[:, k, m * 128:(m + 1) * 128],
                                         bt[:, k, :],
                                         start=(k == 0), stop=(k == KI - 1))
                    th = thP.tile([128, 512], BF16, tag=f"th{m}")
                    for h in range(2):
                        bidx = 2 * p + h
                        nc.scalar.activation(th[:, h * 256:(h + 1) * 256],
                                             ps[:, h * 256:(h + 1) * 256],
                                             AF.Tanh,
                                             bias=ppt[:, m, bidx:bidx + 1],
                                             scale=1.0)
                    ths.append(th)
                pe = psE.tile([1, 512], F32, tag="e")
                for m in range(KH):
                    nc.tensor.matmul(pe[:], wsc[:, m:m + 1], ths[m][:],
                                     start=(m == 0), stop=(m == KH - 1))
                # stage e row -> [2, 256] rows, softmax per pair
                es = smP.tile([1, 512], F32, tag="es")
                nc.vector.tensor_copy(es[:], pe[:])
                ep = smP.tile([2, 256], F32, tag="ep")
                nc.gpsimd.dma_start(out=ep[:], in_=es[:])
                negmax = smP.tile([2, 1], F32, tag="negmax")
                nc.vector.tensor_reduce(negmax[:], ep[:], axis=AX.XYZW,
                                        op=ALU.max, negate=True)
                expd = smP.tile([2, 256], F32, tag="expd")
                esum = smP.tile([2, 1], F32, tag="esum")
                nc.scalar.activation(expd[:], ep[:], AF.Exp, bias=negmax[:],
                                     scale=1.0, accum_out=esum[:])
                rsum = smP.tile([2, 1], F32, tag="rsum")
                nc.vector.reciprocal(rsum[:], esum[:])
                anb = smP.tile([2, 256], BF16, tag="anb")
                anbs[p] = anb
                nc.vector.tensor_scalar_mul(anb[:], expd[:], rsum[:])
                af32 = smP.tile([2, 256], F32, tag="af32")
                nc.gpsimd.tensor_copy(af32[:], anb[:])
                nc.gpsimd.dma_start(out=d["alpha"].ap()[2 * p:2 * p + 2, :],
                                    in_=af32[:])

            def back(q):
                bt, anb = bts.pop(q), anbs.pop(q)
                # broadcast alpha pair to 128 partitions: [128, 2b x 256]
                pbc = psBC.tile([128, 512], F32, tag="pbc")
                nc.tensor.matmul(pbc[:, 0:256], sel[:, 0:128], anb[:],
                                 start=True, stop=True)
                nc.tensor.matmul(pbc[:, 256:512], sel[:, 128:256], anb[:],
                                 start=True, stop=True)
                # context^T: multiply all 4 IN-chunks by alpha, reduce over T
                tmp = tmpP.tile([128, KI, 512], F32, tag="ctmp")
                nc.vector.tensor_mul(
                    tmp[:], bt[:],
                    pbc[:].rearrange("p (o x) -> p o x", o=1)
                         .to_broadcast((128, KI, 512)))
                nc.vector.tensor_reduce(
                    ctxt[:, :, 2 * q:2 * q + 2],
                    tmp[:].rearrange("p k (h t) -> p k h t", h=2),
                    axis=AX.X, op=ALU.add)

            for p in range(NP + 1):
                if p < NP:
                    front(p)
                if p >= 1:
                    back(p - 1)

        # ---- Phase E: LSTM gates + cell update ----
        nc.vector.tensor_copy(inpt[:], ctxt[:])
        with tc.tile_pool(name="wE", bufs=3) as wE, \
             tc.tile_pool(name="psG", bufs=4, space="PSUM") as psG, \
             tc.tile_pool(name="lst", bufs=1) as lst:
            gate_ps = []
            for n in range(4):
                ps_g = psG.tile([S, 512], F32, tag="g")
                wta = wE.tile([128, KI, 512], BF16, tag="wga")
                nc.sync.dma_start(
                    out=wta[:],
                    in_=d["wiht"].ap()[0:IN, n * 512:(n + 1) * 512]
                        .rearrange("(k p) x -> p k x", p=128))
                wtb = wE.tile([NE + 1, 512], BF16, tag="wgb")
                nc.sync.dma_start(out=wtb[:],
                                  in_=d["wiht"].ap()[IN:IN + NE + 1,
                                                     n * 512:(n + 1) * 512])
                wtc = wE.tile([128, KH, 512], BF16, tag="wgc")
                nc.sync.dma_start(
                    out=wtc[:],
                    in_=d["whht"].ap()[:, n * 512:(n + 1) * 512]
                        .rearrange("(k p) x -> p k x", p=128))
                for k in range(KI):
                    nc.tensor.matmul(ps_g[:], inpt[:, k, :], wta[:, k, :],
                                     start=(k == 0), stop=False)
                nc.tensor.matmul(ps_g[:], oh1[:], wtb[:],
                                 start=False, stop=False)
                for k in range(KH):
                    nc.tensor.matmul(ps_g[:], pht[:, k, :], wtc[:, k, :],
                                     start=False, stop=(k == KH - 1))
                gate_ps.append(ps_g)

            i_s = lst.tile([S, 512], F32, tag="i_s")
            f_s = lst.tile([S, 512], F32, tag="f_s")
            g_t = lst.tile([S, 512], F32, tag="g_t")
            o_s = lst.tile([S, 512], F32, tag="o_s")
            nc.scalar.activation(i_s[:], gate_ps[0][:], AF.Sigmoid)
            nc.scalar.activation(f_s[:], gate_ps[1][:], AF.Sigmoid)
            nc.scalar.activation(g_t[:], gate_ps[2][:], AF.Tanh)
            nc.scalar.activation(o_s[:], gate_ps[3][:], AF.Sigmoid)
            t1 = lst.tile([S, 512], F32, tag="t1")
            t2 = lst.tile([S, 512], F32, tag="t2")
            newc = lst.tile([S, 512], F32, tag="newc")
            nc.vector.tensor_mul(t1[:], f_s[:], pc_sb[:])
            nc.vector.tensor_mul(t2[:], i_s[:], g_t[:])
            nc.vector.tensor_add(newc[:], t1[:], t2[:])
            nc.sync.dma_start(out=d["newc"].ap()[:], in_=newc[:])
            tcn = lst.tile([S, 512], F32, tag="tcn")
            nc.scalar.activation(tcn[:], newc[:], AF.Tanh)
            newh = lst.tile([S, 512], F32, tag="newh")
            nc.vector.tensor_mul(newh[:], o_s[:], tcn[:])
            nc.sync.dma_start(out=d["newh"].ap()[:], in_=newh[:])

    nc.compile()
    return nc


_NC_CACHE = None


def _get_nc():
    global _NC_CACHE
    if _NC_CACHE is None:
        _NC_CACHE = _build()
    return _NC_CACHE


def _prep_inputs(prev_h, prev_c, batch_H, char_onehots,
                 W_i2h, W_h2h, b_h2h, W_score, W_ih, W_hh, b_ih, b_hh):
    """Host-side sharding + layout transforms. Returns list of per-core dicts."""
    f32 = np.float32
    bht_all = np.ascontiguousarray(
        batch_H.astype(_bf16).reshape(NCORES, NP, 2, T, IN)
        .transpose(0, 1, 4, 2, 3).reshape(NCORES, NP, IN, 512))
    prevht_all = np.ascontiguousarray(
        prev_h.astype(_bf16).reshape(NCORES, S, KH, 128).transpose(0, 3, 2, 1))
    prevc_all = prev_c.astype(f32).reshape(NCORES, S, H)
    ones = np.ones((NCORES, 1, S), _bf16)
    oh1t_all = np.concatenate(
        [np.ascontiguousarray(
            char_onehots.astype(_bf16).reshape(NCORES, S, NE).transpose(0, 2, 1)),
         ones], axis=1)

    wi2ht = np.ascontiguousarray(W_i2h.T).astype(_bf16)
    wscore = np.ascontiguousarray(W_score[0].reshape(KH, 128).T).astype(_bf16)
    wh2ht = np.ascontiguousarray(W_h2h.T).astype(_bf16)
    bh2h_c = np.ascontiguousarray(b_h2h.reshape(KH, 128).T).astype(f32)
    wiht = np.concatenate(
        [np.ascontiguousarray(W_ih[:, :IN].T),
         np.ascontiguousarray(W_ih[:, IN:].T),
         (b_ih + b_hh)[None, :]], axis=0).astype(_bf16)
    whht = np.ascontiguousarray(W_hh.T).astype(_bf16)

    return [{
        "bht": np.ascontiguousarray(bht_all[c]),
        "prevht": np.ascontiguousarray(prevht_all[c]),
        "prevc": np.ascontiguousarray(prevc_all[c]),
        "oh1t": np.ascontiguousarray(oh1t_all[c]),
        "wi2ht": wi2ht,
        "wscore": wscore,
        "wh2ht": wh2ht,
        "bh2h": bh2h_c,
        "wiht": wiht,
        "whht": whht,
    } for c in range(NCORES)]


def _run(inputs, trace=False):
    nc = _get_nc()
    in_maps = _prep_inputs(**{k: np.asarray(v) for k, v in inputs.items()})
    res = run_bass_kernel_spmd(nc, in_maps, core_ids=list(range(NCORES)),
                               trace=trace)
    new_h = np.concatenate([res.results[c]["newh"] for c in range(NCORES)], 0)
    new_c = np.concatenate([res.results[c]["newc"] for c in range(NCORES)], 0)
    alpha = np.concatenate([res.results[c]["alpha"] for c in range(NCORES)], 0)
    return (new_h.astype(np.float32), new_c.astype(np.float32),
            alpha.astype(np.float32)[:, :, None]), res


def kernel(**inputs):
    out, _ = _run(inputs, trace=False)
    return out
